# revision 1
# baseline (speedup 1.0000x reference)
"""Single-head causal attention on 8 TRN2 NeuronCores, batch-parallel.

Problem: x[8,2048,1024] f32, Wq/Wk/Wv[1024,64] f32
  q,k,v = x@W*  ;  scores = q k^T / sqrt(1024), causal  ;  out = softmax(scores) @ v

Sharding: batch dim across 8 cores (1 batch element per core, no collectives).

Per-core dataflow (all matmuls fp32r = tf32-class, 1 cyc/row at N>=256):
  A) xT: PE-transpose x [2048,1024] -> xT [c=128part x 8ct, t=2048]
  B) proj: lhsT=[Wq|Wv] -> qvT psum [128,512] (rows 0:64 qT, 64:128 vT);
     kT separate (M=64).  vT -> PE-transpose -> v_aug [128s,16,65] with ones col.
  C) per t-chunk (512): st_j [s=128, t=512] = kT_j^T q ; exp(st/32) -> wst (f32r);
     causal mask on diagonal tiles (memset + 0/1 triangle multiply);
     psum_o [65,512] += v_aug_j^T wst_j  (row 64 = softmax denominator);
     PE-transpose psum_o -> [128t, 65], out = cols0:64 * recip(col 64) -> HBM.
"""

import numpy as np

import concourse.bacc as bacc
import concourse.mybir as mybir
import concourse.tile as tile
from concourse.bass_utils import run_bass_kernel_spmd

F32 = mybir.dt.float32
F32R = mybir.dt.float32r

B, T, C, H = 8, 2048, 1024, 64
NCT = C // 128          # 8 c-tiles
NTT = T // 128          # 16 t/s-tiles
NCH = T // 512          # 4 t-chunks
SCALE = float(C ** -0.5)

_CACHE = {}


def build():
    nc = bacc.Bacc(name="head_attn")
    x_d = nc.dram_tensor("x", [T, C], F32, kind="ExternalInput")
    wq_d = nc.dram_tensor("Wq", [C, H], F32, kind="ExternalInput")
    wk_d = nc.dram_tensor("Wk", [C, H], F32, kind="ExternalInput")
    wv_d = nc.dram_tensor("Wv", [C, H], F32, kind="ExternalInput")
    id_d = nc.dram_tensor("ident", [128, 128], F32, kind="ExternalInput")
    ih_d = nc.dram_tensor("identhi", [128, 64], F32, kind="ExternalInput")
    tri_d = nc.dram_tensor("tri", [128, 128], F32, kind="ExternalInput")
    msk_d = nc.dram_tensor("masks", [128, 4, 512], F32, kind="ExternalInput")
    one_d = nc.dram_tensor("ones16", [128, 16, 2], F32, kind="ExternalInput")
    out_d = nc.dram_tensor("out", [T, H], F32, kind="ExternalOutput")

    with tile.TileContext(nc) as tc:
        with (
            tc.tile_pool(name="singles", bufs=1) as singles,
            tc.tile_pool(name="stage", bufs=4) as stage,
            tc.tile_pool(name="work", bufs=8) as work,
            tc.tile_pool(name="outp", bufs=4) as outp,
            tc.tile_pool(name="pbig", bufs=4, space="PSUM") as pbig,
            tc.tile_pool(name="pacc", bufs=2, space="PSUM") as pacc,
            tc.tile_pool(name="psmall", bufs=2, space="PSUM") as psmall,
        ):
            # ---- constants / weights
            ident = singles.tile([128, 128], F32R)
            identhi = singles.tile([128, 64], F32R)
            tri = singles.tile([128, 128], F32R)
            nc.sync.dma_start(ident, id_d[:, :].bitcast(F32R))
            nc.sync.dma_start(identhi, ih_d[:, :].bitcast(F32R))
            nc.sync.dma_start(tri, tri_d[:, :].bitcast(F32R))
            masks = singles.tile([128, 4, 512], F32R)
            nc.sync.dma_start(masks, msk_d[:, :, :].bitcast(F32R))

            wqv = singles.tile([128, NCT, 128], F32R)   # [c | ct | (q h, v h)]
            wk = singles.tile([128, NCT, H], F32R)
            nc.sync.dma_start(
                wqv[:, :, 0:H], wq_d.rearrange("(t c) h -> c t h", c=128).bitcast(F32R))
            nc.sync.dma_start(
                wqv[:, :, H:128], wv_d.rearrange("(t c) h -> c t h", c=128).bitcast(F32R))
            nc.sync.dma_start(
                wk[:, :, :], wk_d.rearrange("(t c) h -> c t h", c=128).bitcast(F32R))

            xT = singles.tile([128, NCT, T], F32R)      # [c, ct, t]

            # ---- A) transpose x into xT
            for tt in range(NTT):
                xs = stage.tile([128, C], F32R, tag="xs")
                nc.sync.dma_start(xs, x_d[tt * 128:(tt + 1) * 128, :].bitcast(F32R))
                for cg in range(2):                     # 2 groups of 4 c-tiles
                    pt = pbig.tile([128, 512], F32R, tag="big")
                    for k in range(4):
                        ct = cg * 4 + k
                        nc.tensor.transpose(
                            pt[:, k * 128:(k + 1) * 128],
                            xs[:, ct * 128:(ct + 1) * 128], ident)
                    dst = xT[:, cg * 4:(cg + 1) * 4, tt * 128:(tt + 1) * 128]
                    if (2 * tt + cg) % 8 < 5:
                        nc.scalar.copy(dst, pt.rearrange("p (a b) -> p a b", a=4))
                    else:
                        nc.vector.tensor_copy(dst, pt.rearrange("p (a b) -> p a b", a=4))

            # ---- B) projections
            qvT = singles.tile([128, T], F32R)          # rows 0:64 qT, 64:128 vT
            kT = singles.tile([64, T], F32R)
            for i in range(NCH):
                pq = pbig.tile([128, 512], F32, tag="big")
                for ct in range(NCT):
                    nc.tensor.matmul(pq, wqv[:, ct, :], xT[:, ct, i * 512:(i + 1) * 512],
                                     start=(ct == 0), stop=(ct == NCT - 1))
                nc.vector.tensor_copy(qvT[:, i * 512:(i + 1) * 512].bitcast(F32R), pq)
                pk = pbig.tile([64, 512], F32, tag="big")
                for ct in range(NCT):
                    nc.tensor.matmul(pk, wk[:, ct, :], xT[:, ct, i * 512:(i + 1) * 512],
                                     start=(ct == 0), stop=(ct == NCT - 1))
                nc.vector.tensor_copy(kT[:, i * 512:(i + 1) * 512].bitcast(F32R), pk)

            # v_aug [s=128, 16, 66] with two ones columns (66 keeps the
            # fp32r output transpose even-sized and partition-base aligned)
            v_aug = singles.tile([128, NTT, 66], F32R)
            nc.sync.dma_start(v_aug[:, :, 64:66], one_d[:, :, :].bitcast(F32R))
            for s in range(NTT):
                pv = psmall.tile([128, 64], F32R, tag="small")
                nc.tensor.transpose(
                    pv, qvT[64:128, s * 128:(s + 1) * 128], identhi[64:128, :])
                nc.vector.tensor_copy(v_aug[:, s, 0:64], pv)

            # ---- C) attention
            for i in range(NCH):
                po = pacc.tile([66, 512], F32)
                nj = 4 * i + 4
                for j in range(nj):
                    pst = pbig.tile([128, 512], F32, tag="big")
                    nc.tensor.matmul(pst, kT[:, j * 128:(j + 1) * 128],
                                     qvT[0:64, i * 512:(i + 1) * 512],
                                     start=True, stop=True)
                    wst = work.tile([128, 512], F32R, tag="wst")
                    k = j - 4 * i
                    d = 128 * k if k > 0 else 0
                    nc.scalar.activation(wst[:, d:], pst[:, d:],
                                         mybir.ActivationFunctionType.Exp, scale=SCALE)
                    if k >= 0:                           # diagonal: mask the triangle
                        nc.vector.tensor_mul(wst[:, d:d + 128], wst[:, d:d + 128], tri)
                    nc.tensor.matmul(po[:, d:], v_aug[:, j, :], wst[:, d:],
                                     start=(j == 0), stop=(j == nj - 1))

                oT = outp.tile([66, 512], F32R, tag="oT")
                nc.scalar.copy(oT, po)
                for b in range(4):
                    pn = psmall.tile([128, 66], F32R, tag="small")
                    nc.tensor.transpose(pn, oT[:, b * 128:(b + 1) * 128],
                                        ident[0:66, 0:66])
                    rec = outp.tile([128, 1], F32, tag="rec")
                    nc.vector.reciprocal(rec, pn[:, 64:65])
                    ob = outp.tile([128, 64], F32, tag="ob")
                    nc.vector.tensor_scalar_mul(ob, pn[:, 0:64], rec)
                    nc.sync.dma_start(
                        out_d[i * 512 + b * 128: i * 512 + (b + 1) * 128, :], ob)

    nc.compile()
    return nc


def _consts():
    ident = np.eye(128, dtype=np.float32)
    identhi = np.zeros((128, 64), dtype=np.float32)
    identhi[64:128, :] = np.eye(64, dtype=np.float32)
    # tri[p, v] = 1 where v >= p  (valid, upper incl diag in [s, u] coords)
    tri = np.triu(np.ones((128, 128), dtype=np.float32))
    return ident, identhi, tri


def kernel(x, Wq, Wk, Wv, trace=False):
    x = np.ascontiguousarray(np.asarray(x, dtype=np.float32))
    Wq = np.ascontiguousarray(np.asarray(Wq, dtype=np.float32))
    Wk = np.ascontiguousarray(np.asarray(Wk, dtype=np.float32))
    Wv = np.ascontiguousarray(np.asarray(Wv, dtype=np.float32))

    if "nc" not in _CACHE:
        _CACHE["nc"] = build()
    nc = _CACHE["nc"]

    ident, identhi, tri = _consts()
    p = np.arange(128, dtype=np.float32)[:, None]
    u = np.arange(512, dtype=np.float32)[None, :]
    masks = np.stack([(u >= p + 128 * k).astype(np.float32) for k in range(4)], axis=1)
    ones16 = np.ones((128, 16, 2), dtype=np.float32)
    in_maps = [
        {"x": x[b], "Wq": Wq, "Wk": Wk, "Wv": Wv,
         "ident": ident, "identhi": identhi, "tri": tri,
         "masks": masks, "ones16": ones16}
        for b in range(B)
    ]
    try:
        res = run_bass_kernel_spmd(nc, in_maps, core_ids=list(range(B)), trace=trace)
    except ModuleNotFoundError:
        res = run_bass_kernel_spmd(nc, in_maps, core_ids=list(range(B)))
    out = np.stack([r["out"] for r in res.results], axis=0)
    kernel.last_exec_time_ns = res.exec_time_ns
    kernel.last_results = res
    return out



# revision 3
# speedup vs baseline: 1.3021x; 1.3021x over previous
"""Single-head causal attention on 8 TRN2 NeuronCores, batch-parallel.

Problem: x[8,2048,1024] f32, Wq/Wk/Wv[1024,64] f32
  q,k,v = x@W*  ;  scores = q k^T / sqrt(1024), causal  ;  out = softmax(scores) @ v

Sharding: batch dim across 8 cores (1 batch element per core, no collectives).

Per-core dataflow (v1 "natural-out", bf16 datapath):
  - x t-tiles DMA'd in REVERSED chunk order [12..15, 8..11, 4..7, 0..3] so the
    last-arriving chunk gates the fewest score tiles (tail is 7 exps, not 16).
  - per t-tile: PE-transpose x (f32r) -> psum; copy-convert -> xT bf16;
    qk-proj (one [128,128] psum: rows 0:64 q, 64:128 k) -> qkT bf16;
    kT moved to partitions 0:64 via SBUF->SBUF DMA; v-proj natural [t,64].
  - scores: st[s,t] tiles [128,512] via lhsT=kT, rhs=qT (bf16);
    exp on Act in PAIRS ([128,2,512] psum -> wst bf16), tri-mask diag on Pool.
  - out natural: po[t,h] = sum_j wst_j^T @ [v_j | 1]; col 64 = softmax denom;
    epilogue: reciprocal + tensor_scalar_mul (DVE) -> out f32 -> DMA per chunk.
"""

import numpy as np

import concourse.bacc as bacc
import concourse.mybir as mybir
import concourse.tile as tile
from concourse.bass_utils import run_bass_kernel_spmd

F32 = mybir.dt.float32
F32R = mybir.dt.float32r
BF16 = mybir.dt.bfloat16
EXP = mybir.ActivationFunctionType.Exp

B, T, C, H = 8, 2048, 1024, 64
NCT = C // 128          # 8 c-tiles
NTT = T // 128          # 16 t-tiles
SCALE = float(C ** -0.5)

TILE_ORDER = [12, 13, 14, 15, 8, 9, 10, 11, 4, 5, 6, 7, 0, 1, 2, 3]
ARR = {tt: s for s, tt in enumerate(TILE_ORDER)}
QT_READY = {3: 3, 2: 7, 1: 11, 0: 15}   # slot when chunk i's 4 tiles have arrived


def _st_schedule():
    sched = [[] for _ in range(16)]
    for i in range(4):
        for j in range(4 * i + 4):
            sched[max(ARR[j], QT_READY[i])].append((i, j))
    for s in range(16):
        sched[s].sort(key=lambda ij: (-ij[0], ARR[ij[1]]))
    return sched


ST_SCHED = _st_schedule()

_CACHE = {}


def build():
    nc = bacc.Bacc(name="head_attn")
    x_d = nc.dram_tensor("x", [T, C], F32, kind="ExternalInput")
    wq_d = nc.dram_tensor("Wq", [C, H], F32, kind="ExternalInput")
    wk_d = nc.dram_tensor("Wk", [C, H], F32, kind="ExternalInput")
    wv_d = nc.dram_tensor("Wv", [C, H], F32, kind="ExternalInput")
    id_d = nc.dram_tensor("ident", [128, 128], F32, kind="ExternalInput")
    tri_d = nc.dram_tensor("tri", [128, 128], F32, kind="ExternalInput")
    out_d = nc.dram_tensor("out", [T, H], F32, kind="ExternalOutput")

    with tile.TileContext(nc) as tc:
        with (
            tc.tile_pool(name="singles", bufs=1) as singles,
            tc.tile_pool(name="xstage", bufs=3) as xstage,
            tc.tile_pool(name="wstp", bufs=1) as wstp,
            tc.tile_pool(name="outp", bufs=1) as outp,
            tc.tile_pool(name="recp", bufs=2) as recp,
            tc.tile_pool(name="ptr", bufs=2, space="PSUM") as ptr,
            tc.tile_pool(name="pqv", bufs=2, space="PSUM") as pqv,
            tc.tile_pool(name="pst", bufs=2, space="PSUM") as pst,
        ):
            # ---- constants / weights
            identr = singles.tile([128, 128], F32R)
            nc.sync.dma_start(identr, id_d[:, :].bitcast(F32R))

            tri_f = singles.tile([128, 128], F32)
            wqk_f = singles.tile([128, NCT, 128], F32)
            wv_f = singles.tile([128, NCT, H], F32)
            nc.scalar.dma_start(tri_f, tri_d[:, :])
            nc.scalar.dma_start(
                wqk_f[:, :, 0:H], wq_d.rearrange("(t c) h -> c t h", c=128))
            nc.scalar.dma_start(
                wqk_f[:, :, H:128], wk_d.rearrange("(t c) h -> c t h", c=128))
            nc.scalar.dma_start(
                wv_f[:, :, :], wv_d.rearrange("(t c) h -> c t h", c=128))

            trib = singles.tile([128, 128], BF16)
            wqkb = singles.tile([128, NCT, 128], BF16)
            wvb = singles.tile([128, NCT, H], BF16)
            nc.gpsimd.tensor_copy(trib, tri_f)
            nc.gpsimd.tensor_copy(wqkb, wqk_f)
            nc.gpsimd.tensor_copy(wvb, wv_f)

            xT = singles.tile([128, NCT, T], BF16)      # [c, ct, t]
            qkT = singles.tile([128, T], BF16)          # rows 0:64 qT, 64:128 kT
            kTsb = singles.tile([64, T], BF16)          # kT at base partition 0
            v_sb = singles.tile([128, NTT, 66], BF16)   # v natural + ones col 64
            nc.gpsimd.memset(v_sb[:, :, 64:66], 1.0)

            out_v = out_d.rearrange("(c a p) h -> c p a h", a=4, p=128)

            # ---- x prefetch for slots 0,1
            xs_tiles = {}
            for s in (0, 1):
                tt = TILE_ORDER[s]
                xs = xstage.tile([128, C], F32R, tag="xs")
                nc.sync.dma_start(xs, x_d[tt * 128:(tt + 1) * 128, :].bitcast(F32R))
                xs_tiles[s] = xs

            wst_loc = {}
            pend = []
            n_exp = [0]

            def flush_exp():
                if not pend:
                    return
                g = len(pend)
                pt = pst.tile([128, 2, 512], F32, tag="pair")
                for h, (i, j) in enumerate(pend):
                    nc.tensor.matmul(pt[:, h, :],
                                     kTsb[:, j * 128:(j + 1) * 128],
                                     qkT[0:64, i * 512:(i + 1) * 512],
                                     start=True, stop=True)
                wt = wstp.tile([128, 2, 512], BF16, tag=f"w{n_exp[0]}")
                n_exp[0] += 1
                nc.scalar.activation(wt[:, 0:g, :], pt[:, 0:g, :], EXP, scale=SCALE)
                for h, (i, j) in enumerate(pend):
                    wst_loc[(i, j)] = (wt, h)
                    k = j - 4 * i
                    if k >= 0:      # diagonal tile: mask lower triangle
                        nc.gpsimd.tensor_mul(wt[:, h, k * 128:(k + 1) * 128],
                                             wt[:, h, k * 128:(k + 1) * 128], trib)
                pend.clear()

            # ---- main slot loop
            for s in range(16):
                tt = TILE_ORDER[s]
                if s + 2 < 16:
                    nt = TILE_ORDER[s + 2]
                    xs = xstage.tile([128, C], F32R, tag="xs")
                    nc.sync.dma_start(xs, x_d[nt * 128:(nt + 1) * 128, :].bitcast(F32R))
                    xs_tiles[s + 2] = xs
                xs = xs_tiles[s]

                # transpose x tile -> xT (bf16)
                for cg in range(2):
                    pt = ptr.tile([128, 512], F32R, tag="tr")
                    for k in range(4):
                        ct = cg * 4 + k
                        nc.tensor.transpose(
                            pt[:, k * 128:(k + 1) * 128],
                            xs[:, ct * 128:(ct + 1) * 128], identr)
                    dst = xT[:, cg * 4:(cg + 1) * 4, tt * 128:(tt + 1) * 128]
                    src = pt.rearrange("p (a b) -> p a b", a=4)
                    if cg == 1 and s <= 8:
                        nc.scalar.copy(dst, src)
                    else:
                        nc.vector.tensor_copy(dst, src)

                # qk projection for this tile -> qkT bf16
                pq = pqv.tile([128, 128], F32, tag="pqv")
                for ct in range(NCT):
                    nc.tensor.matmul(pq, wqkb[:, ct, :],
                                     xT[:, ct, tt * 128:(tt + 1) * 128],
                                     start=(ct == 0), stop=(ct == NCT - 1))
                nc.vector.tensor_copy(qkT[:, tt * 128:(tt + 1) * 128], pq)

                # v projection (natural layout) -> v_sb
                pv = pqv.tile([128, 128], F32, tag="pqv")
                for ct in range(NCT):
                    nc.tensor.matmul(pv[:, 0:H],
                                     xT[:, ct, tt * 128:(tt + 1) * 128],
                                     wvb[:, ct, :],
                                     start=(ct == 0), stop=(ct == NCT - 1))
                nc.vector.tensor_copy(v_sb[:, tt, 0:H], pv[:, 0:H])

                # move kT rows to base partition 0 (DMA can cross partitions)
                nc.sync.dma_start(kTsb[:, tt * 128:(tt + 1) * 128],
                                  qkT[64:128, tt * 128:(tt + 1) * 128])

                # score tiles released this slot (paired exp on Act)
                for ij in ST_SCHED[s]:
                    pend.append(ij)
                    if len(pend) == 2:
                        flush_exp()
                flush_exp()

            # ---- output: po bursts (all gated on the final chunk anyway)
            done = {c: 0 for c in range(4)}
            ob = {}
            for c in range(4):
                obt = outp.tile([128, 4, H], F32, tag=f"ob{c}", name=f"ob{c}")
                ob[c] = obt
            for tt in TILE_ORDER:
                i, tl = tt // 4, tt % 4
                js = [j for j in TILE_ORDER if j <= tt]
                pp = pqv.tile([128, 128], F32, tag="pqv")
                for n, j in enumerate(js):
                    wt, h = wst_loc[(i, j)]
                    nc.tensor.matmul(pp[:, 0:66],
                                     wt[:, h, tl * 128:(tl + 1) * 128],
                                     v_sb[:, j, 0:66],
                                     start=(n == 0), stop=(n == len(js) - 1))
                rec = recp.tile([128, 1], F32, tag="rec")
                nc.vector.reciprocal(rec, pp[:, 64:65])
                nc.vector.tensor_scalar_mul(ob[i][:, tl, :], pp[:, 0:H], rec)
                done[i] += 1
                if done[i] == 4:
                    nc.sync.dma_start(out_v[i], ob[i])

    nc.compile()
    return nc


def _consts():
    ident = np.eye(128, dtype=np.float32)
    # tri[s, t] = 1 where t >= s (valid region of a diagonal [s,t] tile)
    tri = np.triu(np.ones((128, 128), dtype=np.float32))
    return ident, tri


def kernel(x, Wq, Wk, Wv, trace=False):
    x = np.ascontiguousarray(np.asarray(x, dtype=np.float32))
    Wq = np.ascontiguousarray(np.asarray(Wq, dtype=np.float32))
    Wk = np.ascontiguousarray(np.asarray(Wk, dtype=np.float32))
    Wv = np.ascontiguousarray(np.asarray(Wv, dtype=np.float32))

    if "nc" not in _CACHE:
        _CACHE["nc"] = build()
    nc = _CACHE["nc"]

    ident, tri = _consts()
    in_maps = [
        {"x": x[b], "Wq": Wq, "Wk": Wk, "Wv": Wv, "ident": ident, "tri": tri}
        for b in range(B)
    ]
    try:
        res = run_bass_kernel_spmd(nc, in_maps, core_ids=list(range(B)), trace=trace)
    except ModuleNotFoundError:
        res = run_bass_kernel_spmd(nc, in_maps, core_ids=list(range(B)))
    out = np.stack([r["out"] for r in res.results], axis=0)
    kernel.last_exec_time_ns = res.exec_time_ns
    kernel.last_results = res
    return out
